# revision 10
# baseline (speedup 1.0000x reference)
"""GatedBlock Trainium2 kernel: data-parallel over 8 NeuronCores.

Strategy
--------
The op is a blockwise o3.Linear (x0@W0 | x1@W1 per l=1 component | x2@W2 per
l=2 component) followed by silu/sigmoid gating.  It is memory-bound, so the
kernel is laid out to minimize HBM traffic and keep the TensorE contraction
axis on SBUF partitions:

* Host pre-pass: pad rows to a multiple of 8*512, fold the 1/sqrt(fan_in)
  path norms and the sigmoid gate norm into the weights, permute x columns so
  each irrep block is component-major, cast to bf16 and transpose to a
  feature-major xT [960, rows].  All matmul operands then need zero on-chip
  transposes/strided access.
* Device: pure streaming GEMM.  Weights are the stationary operand, rows run
  along the moving free dim (N=512 per matmul, one full PSUM bank).  ScalarE
  produces the sigmoid gates, VectorE applies gating/silu scaling fused with
  the PSUM->SBUF drain (bf16 output cast included).
* Output is written feature-major bf16 yT [768, rows]; the host transposes,
  un-permutes and upcasts to f32.

bf16 end-to-end rel err vs the f32 reference is ~3.4e-3 (tolerance 2e-2).
"""
import numpy as np
import ml_dtypes

import concourse.bass as bass
import concourse.mybir as mybir
import concourse.tile as tile
from concourse.bass_utils import run_bass_kernel_spmd

BF16 = ml_dtypes.bfloat16

N = 200000
NCORES = 8
ROWS = 25088            # padded rows per core (= 49 * 512)
FB = 2048               # rows per DMA block
SB = 512                # rows per matmul sub-block (one PSUM bank)

MUL0, MUL1, MUL2 = 256, 128, 64
SCALARS, NGATES, MULH = 256, 128, 64
SILU_NORM = 1.6791
SIGMOID_NORM = 1.8484

_DT_BF16 = mybir.dt.bfloat16
_DT_F32 = mybir.dt.float32


def _split_excess_waits(nc: bass.Bass, max_waits: int = 1):
    """Walrus in this env rejects instructions with many sync waits
    ("Too many sync wait commands", CoreV3GenImpl setupSyncWait).  Peel
    excess waits onto preceding same-engine NOPs: the engine blocks on the
    NOP's waits first, so semantics are identical (sem values are
    monotonic)."""
    for fn in nc.m.functions:
        for blk in fn.blocks:
            insts = blk.instructions
            out = []
            changed = False
            for inst in insts:
                si = getattr(inst, "sync_info", None)
                if si is not None and len(si.on_wait) > max_waits:
                    waits = list(si.on_wait)
                    keep = waits[:max_waits]
                    rest = waits[max_waits:]
                    while rest:
                        chunk, rest = rest[:max_waits], rest[max_waits:]
                        nop = mybir.InstNoOp(
                            name=nc.get_next_instruction_name(),
                            sync_info=mybir.SyncInfo(on_wait=chunk, on_update=[]),
                            bass_nofuse=True,
                            engine=inst.engine,
                        )
                        out.append(nop)
                    inst.sync_info = mybir.SyncInfo(
                        on_wait=keep, on_update=list(si.on_update)
                    )
                    changed = True
                out.append(inst)
            if changed:
                blk.instructions = out


def build_nc(rows: int = ROWS, fb: int = FB, sb: int = SB, repeat: int = 1) -> bass.Bass:
    nc = bass.Bass()
    xT = nc.declare_dram_parameter("xT", [960, rows], _DT_BF16, isOutput=False)
    w0 = nc.declare_dram_parameter("w0", [256, 384], _DT_BF16, isOutput=False)
    w1 = nc.declare_dram_parameter("w1", [128, 64], _DT_BF16, isOutput=False)
    w2 = nc.declare_dram_parameter("w2", [64, 64], _DT_BF16, isOutput=False)
    yT = nc.declare_dram_parameter("yT", [768, rows], _DT_BF16, isOutput=True)

    mult = mybir.AluOpType.mult
    Sigmoid = mybir.ActivationFunctionType.Sigmoid

    with tile.TileContext(nc) as tc:
        with (
            tc.tile_pool(name="wpool", bufs=1) as wpool,
            tc.tile_pool(name="inpool", bufs=2) as inpool,
            tc.tile_pool(name="outpool", bufs=2) as outpool,
            tc.tile_pool(name="gpool", bufs=2) as gpool,
            tc.tile_pool(name="psum", bufs=1, space=bass.MemorySpace.PSUM) as psum,
        ):
            # --- weights (loaded once) ---
            w0t = wpool.tile([128, 2, 384], _DT_BF16)
            nc.sync.dma_start(w0t[:], w0[:].rearrange("(k p) m -> p k m", p=128))
            w1t = wpool.tile([128, 64], _DT_BF16)
            nc.sync.dma_start(w1t[:], w1[:])
            # W2 duplicated into both partition halves so lhsT base_partition
            # can match rhs slices living at partitions 0-63 or 64-127.
            w2t = wpool.tile([128, 64], _DT_BF16)
            nc.sync.dma_start(w2t[0:64, :], w2[:])
            nc.sync.dma_start(w2t[64:128, :], w2[:])

            for _rep in range(repeat):
              n0 = 0
              while n0 < rows:
                f = min(fb, rows - n0)
                nsub = f // sb
                assert f % sb == 0

                # --- input DMAs: feature-major bf16 chunks ---
                tinA = inpool.tile([128, 2, f], _DT_BF16, tag="tinA")  # x0
                nc.sync.dma_start(
                    tinA[:], xT[0:256, n0:n0 + f].rearrange("(k p) n -> p k n", p=128)
                )
                tinB = inpool.tile([128, 3, f], _DT_BF16, tag="tinB")  # x1 c-major
                nc.sync.dma_start(
                    tinB[:], xT[256:640, n0:n0 + f].rearrange("(k p) n -> p k n", p=128)
                )
                tinC = inpool.tile([128, 2, f], _DT_BF16, tag="tinC")  # x2 c0..c3
                nc.sync.dma_start(
                    tinC[:], xT[640:896, n0:n0 + f].rearrange("(k p) n -> p k n", p=128)
                )
                tinD = inpool.tile([64, f], _DT_BF16, tag="tinD")      # x2 c4
                nc.sync.dma_start(tinD[:], xT[896:960, n0:n0 + f])

                tout0 = outpool.tile([128, 2, f], _DT_BF16, tag="tout0")  # scalars
                tout1 = outpool.tile([128, 2, f], _DT_BF16, tag="tout1")  # y1c0|y2c0, y1c1|y2c1
                tout2 = outpool.tile([128, 2, f], _DT_BF16, tag="tout2")  # y1c2|y2c2, y2c3|y2c4

                for s in range(nsub):
                    sl = slice(s * sb, (s + 1) * sb)
                    x0c0 = tinA[:, 0, sl]
                    x0c1 = tinA[:, 1, sl]
                    x1c = [tinB[:, c, sl] for c in range(3)]
                    x2c = [
                        tinC[0:64, 0, sl], tinC[64:128, 0, sl],
                        tinC[0:64, 1, sl], tinC[64:128, 1, sl],
                        tinD[:, sl],
                    ]

                    psg = psum.tile([128, sb], _DT_F32, tag="psg")
                    ps0 = psum.tile([128, sb], _DT_F32, tag="ps0")
                    ps1 = psum.tile([128, sb], _DT_F32, tag="ps1")
                    pab = psum.tile([128, sb], _DT_F32, tag="pab")
                    pcd = psum.tile([128, sb], _DT_F32, tag="pcd")
                    pef = psum.tile([128, sb], _DT_F32, tag="pef")
                    pgh = psum.tile([128, sb], _DT_F32, tag="pgh")

                    # --- y0 gates first so ScalarE can start early ---
                    nc.tensor.matmul(psg[:], w0t[:, 0, 256:384], x0c0, start=True, stop=False)
                    nc.tensor.matmul(psg[:], w0t[:, 1, 256:384], x0c1, start=False, stop=True)
                    nc.tensor.matmul(ps0[:], w0t[:, 0, 0:128], x0c0, start=True, stop=False)
                    nc.tensor.matmul(ps0[:], w0t[:, 1, 0:128], x0c1, start=False, stop=True)
                    nc.tensor.matmul(ps1[:], w0t[:, 0, 128:256], x0c0, start=True, stop=False)
                    nc.tensor.matmul(ps1[:], w0t[:, 1, 128:256], x0c1, start=False, stop=True)
                    # --- y1 (W1 stationary x3), y2 (W2 stationary x5) ---
                    nc.tensor.matmul(pab[0:64, :], w1t[:], x1c[0])
                    nc.tensor.matmul(pcd[0:64, :], w1t[:], x1c[1])
                    nc.tensor.matmul(pef[0:64, :], w1t[:], x1c[2])
                    nc.tensor.matmul(pab[64:128, :], w2t[0:64, :], x2c[0])
                    nc.tensor.matmul(pcd[64:128, :], w2t[64:128, :], x2c[1])
                    nc.tensor.matmul(pef[64:128, :], w2t[0:64, :], x2c[2])
                    nc.tensor.matmul(pgh[0:64, :], w2t[64:128, :], x2c[3])
                    nc.tensor.matmul(pgh[64:128, :], w2t[0:64, :], x2c[4])

                    # --- ScalarE: sigmoids ---
                    g = gpool.tile([128, sb], _DT_BF16, tag="g")      # [g1; g2]
                    g22 = gpool.tile([128, sb], _DT_BF16, tag="g22")  # [g2; g2]
                    sg0 = gpool.tile([128, sb], _DT_BF16, tag="sg0")
                    sg1 = gpool.tile([128, sb], _DT_BF16, tag="sg1")
                    nc.scalar.activation(g[:], psg[:], Sigmoid)
                    nc.scalar.activation(g22[0:64, :], psg[64:128, :], Sigmoid)
                    nc.scalar.activation(g22[64:128, :], psg[64:128, :], Sigmoid)
                    nc.scalar.activation(sg0[:], ps0[:], Sigmoid)
                    nc.scalar.activation(sg1[:], ps1[:], Sigmoid)

                    # --- VectorE: silu scaling + gating, fused with PSUM drain ---
                    nc.vector.scalar_tensor_tensor(
                        tout0[:, 0, sl], ps0[:], SILU_NORM, sg0[:], mult, mult
                    )
                    nc.vector.scalar_tensor_tensor(
                        tout0[:, 1, sl], ps1[:], SILU_NORM, sg1[:], mult, mult
                    )
                    nc.vector.tensor_tensor(tout1[:, 0, sl], pab[:], g[:], mult)
                    nc.vector.tensor_tensor(tout1[:, 1, sl], pcd[:], g[:], mult)
                    nc.vector.tensor_tensor(tout2[:, 0, sl], pef[:], g[:], mult)
                    nc.vector.tensor_tensor(tout2[:, 1, sl], pgh[:], g22[:], mult)

                # --- output DMAs ---
                nc.sync.dma_start(
                    yT[0:256, n0:n0 + f].rearrange("(k p) n -> p k n", p=128), tout0[:]
                )
                nc.sync.dma_start(
                    yT[256:512, n0:n0 + f].rearrange("(k p) n -> p k n", p=128), tout1[:]
                )
                nc.sync.dma_start(
                    yT[512:768, n0:n0 + f].rearrange("(k p) n -> p k n", p=128), tout2[:]
                )
                n0 += f
    return nc


# --- host-side layout helpers -------------------------------------------------

def _in_perm() -> np.ndarray:
    """column permutation: shuffled feature index -> original x column"""
    p = np.empty(960, dtype=np.int64)
    p[0:256] = np.arange(256)
    # l=1: new 256 + c*128 + m  <- orig 256 + m*3 + c
    c, m = np.meshgrid(np.arange(3), np.arange(128), indexing="ij")
    p[256:640] = (256 + m * 3 + c).reshape(-1)
    # l=2: new 640 + c*64 + m  <- orig 640 + m*5 + c
    c, m = np.meshgrid(np.arange(5), np.arange(64), indexing="ij")
    p[640:960] = (640 + m * 5 + c).reshape(-1)
    return p


def _out_perm() -> np.ndarray:
    """reference output column -> kernel yT row"""
    q = np.empty(768, dtype=np.int64)
    q[0:256] = np.arange(256)
    # kernel rows: 256+  [y1c0(64) y2c0(64) y1c1 y2c1 y1c2 y2c2 y2c3 y2c4]
    y1row = {0: 256, 1: 384, 2: 512}           # y1 component c -> row base
    y2row = {0: 320, 1: 448, 2: 576, 3: 640, 4: 704}
    k = np.arange(64)
    for c in range(3):
        q[256 + k * 3 + c] = y1row[c] + k
    for c in range(5):
        q[448 + k * 5 + c] = y2row[c] + k
    return q


_IN_PERM = _in_perm()
_OUT_PERM = _out_perm()
_NC_CACHE: dict[int, bass.Bass] = {}


def _get_nc() -> bass.Bass:
    if 0 not in _NC_CACHE:
        nc = build_nc()
        _split_excess_waits(nc)  # HW-compile only; CoreSim chokes on the NOPs
        _NC_CACHE[0] = nc
    return _NC_CACHE[0]


def prep_inputs(x, W0, W1, W2):
    """Build the per-core input maps (host-side layout pass)."""
    x = np.asarray(x, dtype=np.float32)
    w0 = (np.asarray(W0, np.float32) / np.sqrt(MUL0)).astype(BF16)
    w1 = (np.asarray(W1, np.float32) * (SIGMOID_NORM / np.sqrt(MUL1))).astype(BF16)
    w2 = (np.asarray(W2, np.float32) * (SIGMOID_NORM / np.sqrt(MUL2))).astype(BF16)

    n = x.shape[0]
    total = NCORES * ROWS
    # shuffled, transposed, padded bf16 xT [960, total]
    xT = np.zeros((960, total), dtype=BF16)
    xT[:, :n] = x[:, _IN_PERM].T
    in_maps = [
        {
            "xT": np.ascontiguousarray(xT[:, c * ROWS:(c + 1) * ROWS]),
            "w0": w0, "w1": w1, "w2": w2,
        }
        for c in range(NCORES)
    ]
    return in_maps


def post_outputs(results, n=N):
    """Gather per-core yT outputs into the full row-major f32 output."""
    yT = np.concatenate([r["yT"] for r in results], axis=1)  # [768, total]
    y = yT[:, :n].astype(np.float32).T                        # [n, 768]
    return np.ascontiguousarray(y[:, _OUT_PERM])


class PjrtRunner:
    """Compile a Bass program once into a reusable sharded PJRT callable.

    Mirrors bass2jax.run_bass_via_pjrt but keeps the jitted executable and
    lets callers hold inputs on device — needed for repeat-timing since the
    axon NTFF profile hook is unavailable in this image.
    """

    def __init__(self, nc: bass.Bass, n_cores: int = NCORES):
        import jax
        from jax.sharding import Mesh, PartitionSpec
        from jax.experimental.shard_map import shard_map
        from concourse import bass2jax, mybir as _mybir

        bass2jax.install_neuronx_cc_hook()
        self.jax = jax
        self.nc = nc
        self.n_cores = n_cores

        partition_name = (
            nc.partition_id_tensor.name if nc.partition_id_tensor else None
        )
        in_names, out_names, out_avals, zero_outs = [], [], [], []
        for alloc in nc.m.functions[0].allocations:
            if not isinstance(alloc, _mybir.MemoryLocationSet):
                continue
            name = alloc.memorylocations[0].name
            if alloc.kind == "ExternalInput":
                if name != partition_name:
                    in_names.append(name)
            elif alloc.kind == "ExternalOutput":
                out_names.append(name)
                shape = tuple(alloc.tensor_shape)
                dtype = _mybir.dt.np(alloc.dtype)
                out_avals.append(jax.core.ShapedArray(shape, dtype))
                zero_outs.append(np.zeros(shape, dtype))
        self.in_names = list(in_names)
        self.out_names = out_names
        self.out_shapes = [tuple(a.shape) for a in out_avals]
        n_params = len(in_names)
        all_in = in_names + out_names
        if partition_name is not None:
            all_in = all_in + [partition_name]

        def _body(*args):
            operands = list(args)
            if partition_name is not None:
                operands.append(bass2jax.partition_id_tensor())
            outs = bass2jax._bass_exec_p.bind(
                *operands,
                out_avals=tuple(out_avals),
                in_names=tuple(all_in),
                out_names=tuple(out_names),
                lowering_input_output_aliases=(),
                sim_require_finite=True,
                sim_require_nnan=True,
                nc=nc,
            )
            return tuple(outs)

        devices = jax.devices()[:n_cores]
        self.mesh = Mesh(np.asarray(devices), ("core",))
        in_specs = (PartitionSpec("core"),) * (n_params + len(out_names))
        out_specs = (PartitionSpec("core"),) * len(out_names)
        # no donation: we reuse the zero output buffers across timed calls
        self.fn = jax.jit(
            shard_map(_body, mesh=self.mesh, in_specs=in_specs,
                      out_specs=out_specs, check_rep=False),
            keep_unused=True,
        )
        self.zero_outs = zero_outs

    def put_inputs(self, in_maps):
        """device_put concatenated per-core inputs; returns device args."""
        import jax
        from jax.sharding import NamedSharding, PartitionSpec
        args = []
        for i, name in enumerate(self.in_names):
            arr = np.concatenate([np.asarray(m[name]) for m in in_maps], axis=0)
            args.append(jax.device_put(
                arr, NamedSharding(self.mesh, PartitionSpec("core"))))
        for z in self.zero_outs:
            zz = np.concatenate([z] * self.n_cores, axis=0)
            args.append(jax.device_put(
                zz, NamedSharding(self.mesh, PartitionSpec("core"))))
        return args

    def exec_only(self, dev_args):
        outs = self.fn(*dev_args)
        self.jax.block_until_ready(outs)
        return outs

    def __call__(self, in_maps):
        outs = self.exec_only(self.put_inputs(in_maps))
        res = []
        for c in range(self.n_cores):
            d = {}
            for i, name in enumerate(self.out_names):
                full = np.asarray(outs[i])
                d[name] = full.reshape(self.n_cores, *self.out_shapes[i])[c]
            res.append(d)
        return res


_RUNNER_CACHE: dict = {}


def get_runner(repeat: int = 1) -> PjrtRunner:
    if repeat not in _RUNNER_CACHE:
        nc = build_nc(repeat=repeat)
        _split_excess_waits(nc)
        _RUNNER_CACHE[repeat] = PjrtRunner(nc)
    return _RUNNER_CACHE[repeat]


def run(x, W0, W1, W2, **kw):
    runner = get_runner()
    in_maps = prep_inputs(x, W0, W1, W2)
    results = runner(in_maps)
    return post_outputs(results), results


def kernel(x, W0, W1, W2):
    out, _ = run(x, W0, W1, W2)
    return out


# revision 14
# speedup vs baseline: 21.9310x; 21.9310x over previous
"""GatedBlock Trainium2 kernel: data-parallel over 8 NeuronCores.

Strategy
--------
The op is a blockwise o3.Linear (x0@W0 | x1@W1 per l=1 component | x2@W2 per
l=2 component) followed by silu/sigmoid gating.  It is memory-bound, so the
kernel is laid out to minimize HBM traffic and keep the TensorE contraction
axis on SBUF partitions:

* Host pre-pass: pad rows to a multiple of 8*512, fold the 1/sqrt(fan_in)
  path norms and the sigmoid gate norm into the weights, permute x columns so
  each irrep block is component-major, cast to bf16 and transpose to a
  feature-major xT [960, rows].  All matmul operands then need zero on-chip
  transposes/strided access.
* Device: pure streaming GEMM.  Weights are the stationary operand, rows run
  along the moving free dim (N=512 per matmul, one full PSUM bank).  ScalarE
  produces the sigmoid gates, VectorE applies gating/silu scaling fused with
  the PSUM->SBUF drain (bf16 output cast included).
* Output is written feature-major bf16 yT [768, rows]; the host transposes,
  un-permutes and upcasts to f32.

bf16 end-to-end rel err vs the f32 reference is ~3.4e-3 (tolerance 2e-2).
"""
import numpy as np
import ml_dtypes

import concourse.bass as bass
import concourse.mybir as mybir
import concourse.tile as tile
from concourse.bass_utils import run_bass_kernel_spmd

BF16 = ml_dtypes.bfloat16

N = 200000
NCORES = 8
ROWS = 25088            # padded rows per core (= 49 * 512)
FB = 2048               # rows per DMA block
SB = 512                # rows per matmul sub-block (one PSUM bank)

MUL0, MUL1, MUL2 = 256, 128, 64
SCALARS, NGATES, MULH = 256, 128, 64
SILU_NORM = 1.6791
SIGMOID_NORM = 1.8484

_DT_BF16 = mybir.dt.bfloat16
_DT_F32 = mybir.dt.float32


def _split_excess_waits(nc: bass.Bass, max_waits: int = 1):
    """Walrus in this env rejects instructions with many sync waits
    ("Too many sync wait commands", CoreV3GenImpl setupSyncWait).  Peel
    excess waits onto preceding same-engine NOPs: the engine blocks on the
    NOP's waits first, so semantics are identical (sem values are
    monotonic)."""
    for fn in nc.m.functions:
        for blk in fn.blocks:
            insts = blk.instructions
            out = []
            changed = False
            for inst in insts:
                si = getattr(inst, "sync_info", None)
                if si is not None and len(si.on_wait) > max_waits:
                    waits = list(si.on_wait)
                    keep = waits[:max_waits]
                    rest = waits[max_waits:]
                    while rest:
                        chunk, rest = rest[:max_waits], rest[max_waits:]
                        nop = mybir.InstNoOp(
                            name=nc.get_next_instruction_name(),
                            sync_info=mybir.SyncInfo(on_wait=chunk, on_update=[]),
                            bass_nofuse=True,
                            engine=inst.engine,
                        )
                        out.append(nop)
                    inst.sync_info = mybir.SyncInfo(
                        on_wait=keep, on_update=list(si.on_update)
                    )
                    changed = True
                out.append(inst)
            if changed:
                blk.instructions = out


def build_nc(rows: int = ROWS, fb: int = FB, sb: int = SB, repeat: int = 1) -> bass.Bass:
    nc = bass.Bass()
    xT = nc.declare_dram_parameter("xT", [960, rows], _DT_BF16, isOutput=False)
    w0 = nc.declare_dram_parameter("w0", [256, 384], _DT_BF16, isOutput=False)
    w1 = nc.declare_dram_parameter("w1", [128, 64], _DT_BF16, isOutput=False)
    w2 = nc.declare_dram_parameter("w2", [64, 64], _DT_BF16, isOutput=False)
    yT = nc.declare_dram_parameter("yT", [768, rows], _DT_BF16, isOutput=True)

    mult = mybir.AluOpType.mult
    Sigmoid = mybir.ActivationFunctionType.Sigmoid

    with tile.TileContext(nc) as tc:
        with (
            tc.tile_pool(name="wpool", bufs=1) as wpool,
            tc.tile_pool(name="inpool", bufs=3) as inpool,
            tc.tile_pool(name="outpool", bufs=2) as outpool,
            tc.tile_pool(name="gpool", bufs=2) as gpool,
            tc.tile_pool(name="psum", bufs=1, space=bass.MemorySpace.PSUM) as psum,
        ):
            # --- weights (loaded once) ---
            w0t = wpool.tile([128, 2, 384], _DT_BF16)
            nc.sync.dma_start(w0t[:], w0[:].rearrange("(k p) m -> p k m", p=128))
            w1t = wpool.tile([128, 64], _DT_BF16)
            nc.sync.dma_start(w1t[:], w1[:])
            # W2 duplicated into both partition halves so lhsT base_partition
            # can match rhs slices living at partitions 0-63 or 64-127.
            w2t = wpool.tile([128, 64], _DT_BF16)
            nc.sync.dma_start(w2t[0:64, :], w2[:])
            nc.sync.dma_start(w2t[64:128, :], w2[:])

            for _rep in range(repeat):
              n0 = 0
              while n0 < rows:
                f = min(fb, rows - n0)
                nsub = f // sb
                assert f % sb == 0

                # --- input DMAs: feature-major bf16 chunks ---
                tinA = inpool.tile([128, 2, f], _DT_BF16, tag="tinA")  # x0
                nc.sync.dma_start(
                    tinA[:], xT[0:256, n0:n0 + f].rearrange("(k p) n -> p k n", p=128)
                )
                tinB = inpool.tile([128, 3, f], _DT_BF16, tag="tinB")  # x1 c-major
                nc.sync.dma_start(
                    tinB[:], xT[256:640, n0:n0 + f].rearrange("(k p) n -> p k n", p=128)
                )
                tinC = inpool.tile([128, 2, f], _DT_BF16, tag="tinC")  # x2 c0..c3
                nc.sync.dma_start(
                    tinC[:], xT[640:896, n0:n0 + f].rearrange("(k p) n -> p k n", p=128)
                )
                tinD = inpool.tile([64, f], _DT_BF16, tag="tinD")      # x2 c4
                nc.sync.dma_start(tinD[:], xT[896:960, n0:n0 + f])

                tout0 = outpool.tile([128, 2, f], _DT_BF16, tag="tout0")  # scalars
                tout1 = outpool.tile([128, 2, f], _DT_BF16, tag="tout1")  # y1c0|y2c0, y1c1|y2c1
                tout2 = outpool.tile([128, 2, f], _DT_BF16, tag="tout2")  # y1c2|y2c2, y2c3|y2c4

                for s in range(nsub):
                    sl = slice(s * sb, (s + 1) * sb)
                    x0c0 = tinA[:, 0, sl]
                    x0c1 = tinA[:, 1, sl]
                    x1c = [tinB[:, c, sl] for c in range(3)]
                    x2c = [
                        tinC[0:64, 0, sl], tinC[64:128, 0, sl],
                        tinC[0:64, 1, sl], tinC[64:128, 1, sl],
                        tinD[:, sl],
                    ]

                    psg = psum.tile([128, sb], _DT_F32, tag="psg")
                    ps01 = psum.tile([128, 2, sb], _DT_F32, tag="ps01")
                    pab = psum.tile([128, sb], _DT_F32, tag="pab")
                    pcd = psum.tile([128, sb], _DT_F32, tag="pcd")
                    pef = psum.tile([128, sb], _DT_F32, tag="pef")
                    pgh = psum.tile([128, sb], _DT_F32, tag="pgh")

                    # --- y0 gates first so ScalarE can start early ---
                    nc.tensor.matmul(psg[:], w0t[:, 0, 256:384], x0c0, start=True, stop=False)
                    nc.tensor.matmul(psg[:], w0t[:, 1, 256:384], x0c1, start=False, stop=True)
                    nc.tensor.matmul(ps01[:, 0, :], w0t[:, 0, 0:128], x0c0, start=True, stop=False)
                    nc.tensor.matmul(ps01[:, 0, :], w0t[:, 1, 0:128], x0c1, start=False, stop=True)
                    nc.tensor.matmul(ps01[:, 1, :], w0t[:, 0, 128:256], x0c0, start=True, stop=False)
                    nc.tensor.matmul(ps01[:, 1, :], w0t[:, 1, 128:256], x0c1, start=False, stop=True)
                    # --- y1 (W1 stationary x3), y2 (W2 stationary x5) ---
                    nc.tensor.matmul(pab[0:64, :], w1t[:], x1c[0])
                    nc.tensor.matmul(pcd[0:64, :], w1t[:], x1c[1])
                    nc.tensor.matmul(pef[0:64, :], w1t[:], x1c[2])
                    nc.tensor.matmul(pab[64:128, :], w2t[0:64, :], x2c[0])
                    nc.tensor.matmul(pcd[64:128, :], w2t[64:128, :], x2c[1])
                    nc.tensor.matmul(pef[64:128, :], w2t[0:64, :], x2c[2])
                    nc.tensor.matmul(pgh[0:64, :], w2t[64:128, :], x2c[3])
                    nc.tensor.matmul(pgh[64:128, :], w2t[0:64, :], x2c[4])

                    # --- ScalarE: sigmoids (one op for gates, one for scalars) ---
                    g = gpool.tile([128, sb], _DT_BF16, tag="g")        # [g1; g2]
                    sg = gpool.tile([128, 2, sb], _DT_BF16, tag="sg")
                    nc.scalar.activation(g[:], psg[:], Sigmoid)
                    nc.scalar.activation(sg[:], ps01[:], Sigmoid)

                    # --- VectorE: silu scaling + gating, fused with PSUM drain ---
                    nc.vector.scalar_tensor_tensor(
                        tout0[:, :, sl], ps01[:], SILU_NORM, sg[:], mult, mult
                    )
                    nc.vector.tensor_tensor(tout1[:, 0, sl], pab[:], g[:], mult)
                    nc.vector.tensor_tensor(tout1[:, 1, sl], pcd[:], g[:], mult)
                    nc.vector.tensor_tensor(tout2[:, 0, sl], pef[:], g[:], mult)
                    nc.vector.tensor_tensor(
                        tout2[0:64, 1, sl], pgh[0:64, :], g[64:128, :], mult
                    )
                    nc.vector.tensor_tensor(
                        tout2[64:128, 1, sl], pgh[64:128, :], g[64:128, :], mult
                    )

                # --- output DMAs (ACT HWDGE ring, parallel to SP's input ring) ---
                nc.scalar.dma_start(
                    yT[0:256, n0:n0 + f].rearrange("(k p) n -> p k n", p=128), tout0[:]
                )
                nc.scalar.dma_start(
                    yT[256:512, n0:n0 + f].rearrange("(k p) n -> p k n", p=128), tout1[:]
                )
                nc.scalar.dma_start(
                    yT[512:768, n0:n0 + f].rearrange("(k p) n -> p k n", p=128), tout2[:]
                )
                n0 += f
    return nc


# --- host-side layout helpers -------------------------------------------------

def _in_perm() -> np.ndarray:
    """column permutation: shuffled feature index -> original x column"""
    p = np.empty(960, dtype=np.int64)
    p[0:256] = np.arange(256)
    # l=1: new 256 + c*128 + m  <- orig 256 + m*3 + c
    c, m = np.meshgrid(np.arange(3), np.arange(128), indexing="ij")
    p[256:640] = (256 + m * 3 + c).reshape(-1)
    # l=2: new 640 + c*64 + m  <- orig 640 + m*5 + c
    c, m = np.meshgrid(np.arange(5), np.arange(64), indexing="ij")
    p[640:960] = (640 + m * 5 + c).reshape(-1)
    return p


def _out_perm() -> np.ndarray:
    """reference output column -> kernel yT row"""
    q = np.empty(768, dtype=np.int64)
    q[0:256] = np.arange(256)
    # kernel rows: 256+  [y1c0(64) y2c0(64) y1c1 y2c1 y1c2 y2c2 y2c3 y2c4]
    y1row = {0: 256, 1: 384, 2: 512}           # y1 component c -> row base
    y2row = {0: 320, 1: 448, 2: 576, 3: 640, 4: 704}
    k = np.arange(64)
    for c in range(3):
        q[256 + k * 3 + c] = y1row[c] + k
    for c in range(5):
        q[448 + k * 5 + c] = y2row[c] + k
    return q


_IN_PERM = _in_perm()
_OUT_PERM = _out_perm()
_NC_CACHE: dict[int, bass.Bass] = {}


def _get_nc() -> bass.Bass:
    if 0 not in _NC_CACHE:
        nc = build_nc()
        _split_excess_waits(nc)  # HW-compile only; CoreSim chokes on the NOPs
        _NC_CACHE[0] = nc
    return _NC_CACHE[0]


def prep_inputs(x, W0, W1, W2):
    """Build the per-core input maps (host-side layout pass)."""
    x = np.asarray(x, dtype=np.float32)
    w0 = (np.asarray(W0, np.float32) / np.sqrt(MUL0)).astype(BF16)
    w1 = (np.asarray(W1, np.float32) * (SIGMOID_NORM / np.sqrt(MUL1))).astype(BF16)
    w2 = (np.asarray(W2, np.float32) * (SIGMOID_NORM / np.sqrt(MUL2))).astype(BF16)

    n = x.shape[0]
    total = NCORES * ROWS
    # shuffled, transposed, padded bf16 xT [960, total]
    xT = np.zeros((960, total), dtype=BF16)
    xT[:, :n] = x[:, _IN_PERM].T
    in_maps = [
        {
            "xT": np.ascontiguousarray(xT[:, c * ROWS:(c + 1) * ROWS]),
            "w0": w0, "w1": w1, "w2": w2,
        }
        for c in range(NCORES)
    ]
    return in_maps


def post_outputs(results, n=N):
    """Gather per-core yT outputs into the full row-major f32 output."""
    yT = np.concatenate([r["yT"] for r in results], axis=1)  # [768, total]
    y = yT[:, :n].astype(np.float32).T                        # [n, 768]
    return np.ascontiguousarray(y[:, _OUT_PERM])


class PjrtRunner:
    """Compile a Bass program once into a reusable sharded PJRT callable.

    Mirrors bass2jax.run_bass_via_pjrt but keeps the jitted executable and
    lets callers hold inputs on device — needed for repeat-timing since the
    axon NTFF profile hook is unavailable in this image.
    """

    def __init__(self, nc: bass.Bass, n_cores: int = NCORES):
        import jax
        from jax.sharding import Mesh, PartitionSpec
        from jax.experimental.shard_map import shard_map
        from concourse import bass2jax, mybir as _mybir

        bass2jax.install_neuronx_cc_hook()
        self.jax = jax
        self.nc = nc
        self.n_cores = n_cores

        partition_name = (
            nc.partition_id_tensor.name if nc.partition_id_tensor else None
        )
        in_names, out_names, out_avals, zero_outs = [], [], [], []
        for alloc in nc.m.functions[0].allocations:
            if not isinstance(alloc, _mybir.MemoryLocationSet):
                continue
            name = alloc.memorylocations[0].name
            if alloc.kind == "ExternalInput":
                if name != partition_name:
                    in_names.append(name)
            elif alloc.kind == "ExternalOutput":
                out_names.append(name)
                shape = tuple(alloc.tensor_shape)
                dtype = _mybir.dt.np(alloc.dtype)
                out_avals.append(jax.core.ShapedArray(shape, dtype))
                zero_outs.append(np.zeros(shape, dtype))
        self.in_names = list(in_names)
        self.out_names = out_names
        self.out_shapes = [tuple(a.shape) for a in out_avals]
        n_params = len(in_names)
        all_in = in_names + out_names
        if partition_name is not None:
            all_in = all_in + [partition_name]

        def _body(*args):
            operands = list(args)
            if partition_name is not None:
                operands.append(bass2jax.partition_id_tensor())
            outs = bass2jax._bass_exec_p.bind(
                *operands,
                out_avals=tuple(out_avals),
                in_names=tuple(all_in),
                out_names=tuple(out_names),
                lowering_input_output_aliases=(),
                sim_require_finite=True,
                sim_require_nnan=True,
                nc=nc,
            )
            return tuple(outs)

        devices = jax.devices()[:n_cores]
        self.mesh = Mesh(np.asarray(devices), ("core",))
        in_specs = (PartitionSpec("core"),) * (n_params + len(out_names))
        out_specs = (PartitionSpec("core"),) * len(out_names)
        # no donation: we reuse the zero output buffers across timed calls
        self.fn = jax.jit(
            shard_map(_body, mesh=self.mesh, in_specs=in_specs,
                      out_specs=out_specs, check_rep=False),
            keep_unused=True,
        )
        self.zero_outs = zero_outs

    def put_inputs(self, in_maps):
        """device_put concatenated per-core inputs; returns device args."""
        import jax
        from jax.sharding import NamedSharding, PartitionSpec
        args = []
        for i, name in enumerate(self.in_names):
            arr = np.concatenate([np.asarray(m[name]) for m in in_maps], axis=0)
            args.append(jax.device_put(
                arr, NamedSharding(self.mesh, PartitionSpec("core"))))
        for z in self.zero_outs:
            zz = np.concatenate([z] * self.n_cores, axis=0)
            args.append(jax.device_put(
                zz, NamedSharding(self.mesh, PartitionSpec("core"))))
        return args

    def exec_only(self, dev_args):
        outs = self.fn(*dev_args)
        self.jax.block_until_ready(outs)
        return outs

    def __call__(self, in_maps):
        outs = self.exec_only(self.put_inputs(in_maps))
        res = []
        for c in range(self.n_cores):
            d = {}
            for i, name in enumerate(self.out_names):
                full = np.asarray(outs[i])
                d[name] = full.reshape(self.n_cores, *self.out_shapes[i])[c]
            res.append(d)
        return res


_RUNNER_CACHE: dict = {}


def get_runner(repeat: int = 1) -> PjrtRunner:
    if repeat not in _RUNNER_CACHE:
        nc = build_nc(repeat=repeat)
        _split_excess_waits(nc)
        _RUNNER_CACHE[repeat] = PjrtRunner(nc)
    return _RUNNER_CACHE[repeat]


def run(x, W0, W1, W2, **kw):
    runner = get_runner()
    in_maps = prep_inputs(x, W0, W1, W2)
    results = runner(in_maps)
    return post_outputs(results), results


def kernel(x, W0, W1, W2):
    out, _ = run(x, W0, W1, W2)
    return out
